# revision 36
# baseline (speedup 1.0000x reference)
"""Trainium2 Bass kernel for the DNL (disentangled non-local) attention block.

Reference computation (per batch b, with xf = x.reshape(B, C, N), N = H*W):
    q  = (wq @ xf + bq)  centered over n          [N, 32]
    k  = (wk @ xf + bk)  centered over n          [32, N]
    A  = softmax_rows(q @ k)                      [N, N]
    v  = relu(wv @ xf + bv)                       [C, N]
    mask = softmax(wm @ xf + bm)                  [N]
    tissue[c, m] = sum_n v[c, n] * (A[m, n] + mask[n])
    return (x, tissue)

Math simplifications used (all exact):
  - q/k biases, bm, and k-centering add per-row constants inside the row
    softmax and drop out; only q-centering survives (as "-mean_n q").
  - The mask term is a rank-1 correction vm[c] = sum_n v[c,n] mask[n].
  - No max-subtraction in softmax: |energy| <= ~5 for these input scales.

Device layout (per core; 8 cores = 4 batches x 2 query-halves of 2048):
  - E^T[j, m] = K[:, j]^T @ Qc^T[:, m] computed j-partitioned so that the
    softmax denominator and the AV matmul both consume it without transposes.
  - exp(E^T) is written as float8_e4m3; the AV matmul and the denominator
    both run as fp8 DoubleRow matmuls (contract 256 = jb-pairs), halving
    the PE stream cost vs bf16.  Z[m] = colsum rides the PE as a ones-
    stationary DoubleRow matmul accumulated across the whole m-chunk.
  - exp is emitted as TWO [128,1024] ACTIVATEs per 4-jb group into ping-
    ponged 2-bank PSUM tiles, so the next group's E matmuls write one
    half while ACT reads the other: the ACT queue never stalls on the
    E->exp->E chain that cost ~580ns/group with a single 4-bank buffer.
  - The per-core query half is selected by permuting the spatial columns of
    the input on the host (j-sums are permutation invariant).

Schedule notes (from perfetto traces):
  - Weight DMAs are emitted BEFORE the 4MB x DMA (single DMA queue).
  - K and Q project in one packed pass (stationary [128, 64] = wk|wq);
    q-sums for centering fall out of the ACT staging copies via accum_out.
  - pm (mask logits) is computed directly in column layout by reusing the
    V-projection's stationary xf blocks with wm as the moving operand.
  - The per-m-chunk epilogue (1/z, *rz, +vm, DMA out) contains zero PE
    instructions and is emitted lazily one chunk later; only the PSUM-
    freeing copies are emitted eagerly.
"""

import sys

import ml_dtypes
import numpy as np

BF16_NP = ml_dtypes.bfloat16

if "/opt/trn_rl_repo" not in sys.path:
    sys.path.insert(0, "/opt/trn_rl_repo")

import concourse.bacc as bacc
import concourse.bass as bass
import concourse.mybir as mybir
import concourse.tile as tile
from concourse.bass_utils import run_bass_kernel_spmd

F32 = mybir.dt.float32
F32R = mybir.dt.float32r
F16 = mybir.dt.float16
BF16 = mybir.dt.bfloat16
FP8 = mybir.dt.float8e4
AF = mybir.ActivationFunctionType
DR = mybir.MatmulPerfMode.DoubleRow

B, C, H, W = 4, 256, 64, 64
N = H * W          # 4096 spatial positions
D = 32             # C // 8, q/k channel dim
M = N // 2         # query rows per core (2048)
NB = N // 128      # 32 j-blocks
NMC = M // 512     # 4 m-chunks per core
NG = NB // 4       # 8 4-jb groups per m-chunk
N_CORES = 8


def build_nc():
    nc = bacc.Bacc("TRN2", target_bir_lowering=False)

    x_d = nc.dram_tensor("x", [C, N], BF16, kind="ExternalInput")
    wkq_d = nc.dram_tensor("wkq", [128, 2, 2 * D + 1], BF16, kind="ExternalInput")
    wvt_d = nc.dram_tensor("wvt", [128, 2, C], BF16, kind="ExternalInput")
    bv_d = nc.dram_tensor("bv", [1, C], F32, kind="ExternalInput")
    bv2c_d = nc.dram_tensor("bv2c", [128, 2], F32, kind="ExternalInput")
    out_d = nc.dram_tensor("out", [C, M], F32, kind="ExternalOutput")

    with tile.TileContext(nc) as tc, nc.allow_low_precision(
        reason="fp8/bf16 matmul operands are a deliberate precision trade"
    ):
        with (
            tc.tile_pool(name="const", bufs=1) as cpool,
            tc.tile_pool(name="work", bufs=1) as wpool,
            tc.tile_pool(name="norm", bufs=2) as npool,
            tc.tile_pool(name="expsb", bufs=3) as epool,
            tc.tile_pool(name="osb", bufs=2) as opool,
        ):
            # ---------------- DMAs: small weights first ----------------
            wkq = cpool.tile([128, 2, 2 * D + 1], BF16, tag="wkq")
            wvt = cpool.tile([128, 2, C], BF16, tag="wvt")
            bv = cpool.tile([1, C], F32, tag="bv")
            bv2c = cpool.tile([128, 2], F32, tag="bv2c")
            nc.sync.dma_start(wkq[:], wkq_d[:])
            nc.sync.dma_start(bv[:], bv_d[:])
            nc.sync.dma_start(bv2c[:], bv2c_d[:])

            # x lands in PER-CHUNK tiles: with one big tile per c-half the
            # Tile dep tracker made the first kq matmul wait for the whole
            # 4-chunk stream (~5us of dead PE time at the start)
            xts = [
                [
                    cpool.tile(
                        [128, 1024], BF16,
                        tag=f"xf{cb}_{c}", name=f"xf{cb}_{c}",
                    )
                    for c in range(4)
                ]
                for cb in range(2)
            ]
            nc.sync.dma_start(xts[0][0][:], x_d[0:128, bass.ts(0, 1024)])
            nc.sync.dma_start(xts[1][0][:], x_d[128:256, bass.ts(0, 1024)])
            nc.sync.dma_start(wvt[:], wvt_d[:])
            for t in range(1, 4):
                nc.sync.dma_start(xts[0][t][:], x_d[0:128, bass.ts(t, 1024)])
                nc.sync.dma_start(xts[1][t][:], x_d[128:256, bass.ts(t, 1024)])

            def xchunk(cb, col0, ncols):
                c = col0 // 1024
                return xts[cb][c][:, col0 - 1024 * c : col0 - 1024 * c + ncols]

            ones_colf = cpool.tile([128, 1], F32, tag="ones_colf")
            nc.vector.memset(ones_colf[:], 1.0)
            ones_col = cpool.tile([128, 1], BF16, tag="ones_col")
            nc.vector.tensor_copy(ones_col[:], ones_colf[:])
            # ones pair for the DoubleRow Z (denominator) matmuls; padded to
            # 16B pair stride for the Ldweights ISA check
            ones_pair_t = cpool.tile([128, 2, 16], FP8, tag="ones_pair")
            nc.vector.memset(ones_pair_t[:], 1.0)
            ones_pair = ones_pair_t[:, :, 0:1]
            # No HAM warmup: the projections start as soon as x chunk 0
            # lands and their own matmul stream opens the clock gate ~3.4us
            # in; dummy warmup matmuls only delayed the projections.
            ones_rowf = cpool.tile([1, 128], F32, tag="ones_rowf")
            nc.vector.memset(ones_rowf[:], 1.0)
            # identity for the pm-row transposes; lives on partition 64 to
            # match the pm row's base partition
            ones_rowb = cpool.tile([2 * D + 1, 1], BF16, tag="ones_rowb")
            nc.vector.memset(ones_rowb[:], 1.0)
            # -bv broadcast to all partitions: the V relu becomes
            # vt = max(vp, -bv); the missing +bv is exact-folded into the
            # vm_col constant as +2*bv (attention rows and the mask each
            # sum to exactly 1 over n)
            nbv_row = cpool.tile([1, C], F32, tag="nbv_row")
            nc.scalar.mul(nbv_row[:], bv[:], -1.0)
            # broadcast twice: the V relu consumes 2-jb [128, 2, C] tiles
            nbvb = cpool.tile([128, 2, C], F32, tag="nbvb")
            nc.gpsimd.partition_broadcast(nbvb[:, 0, :], nbv_row[:])
            nc.gpsimd.partition_broadcast(nbvb[:, 1, :], nbv_row[:])

            # ---------------- stage A: projections ----------------
            # k rows 0:32 | q rows 32:64 | pm row 64 staged by ONE merged
            # ACTIVATE per 512-col chunk (the old 3-ACTIVATE+2-accum-read
            # split cost ~21us of serial ACT and dominated the prologue)
            kqm_sb = cpool.tile([2 * D + 1, N], BF16, tag="kqm_sb")
            k_rep = cpool.tile([4 * D, N], BF16, tag="k_rep")
            qct = cpool.tile([4 * D, M], BF16, tag="qct")
            qpart = cpool.tile([2 * D + 1, 8], F32, tag="qpart")
            vt_sb = cpool.tile([128, NB, C], FP8, tag="vt_sb")
            mask_col = cpool.tile([128, NB], FP8, tag="mask_col")
            ztt = cpool.tile([1, 1], F32, tag="ztt")
            rz = cpool.tile([1, 1], F32, tag="rz")
            rzc = cpool.tile([128, 1], F32, tag="rzc")
            vm_col = cpool.tile([128, 2], F32, tag="vm_col")
            qsum = wpool.tile([2 * D, 1], F32, tag="qsum")
            qneg = wpool.tile([2 * D, 1], F32, tag="qneg")

            with (
                tc.tile_pool(name="psA", bufs=2, space="PSUM") as psA,
                tc.tile_pool(name="psB", bufs=4, space="PSUM") as psB,
            ):
                # K|Q|pm packed pass (stationary cols 0-31 = wk,
                # 32-63 = wq, 64 = wm) interleaved with the V projection
                # 4 j-blocks per t-chunk: matches the x DMA arrival pace
                # so the PE never idles during the input stream.
                for t in range(8):
                    kq = psA.tile([2 * D + 1, 512], F32, tag="kq_ps")
                    for cb in range(2):
                        nc.tensor.matmul(
                            kq[:],
                            wkq[:, cb, :],
                            xchunk(cb, 512 * t, 512),
                            start=(cb == 0),
                            stop=(cb == 1),
                        )
                    nc.scalar.activation(
                        kqm_sb[:, bass.ts(t, 512)],
                        kq[:],
                        AF.Copy,
                        accum_out=qpart[:, t : t + 1],
                    )
                    # k replicated to all four row groups per chunk, as
                    # independent single-hop DMAs split over two queues
                    # (serial log-doubling at the end cost ~2us of the
                    # projection->attention transition)
                    for h in range(4):
                        eng = nc.sync if h < 2 else nc.gpsimd
                        eng.dma_start(
                            k_rep[h * D : (h + 1) * D, bass.ts(t, 512)],
                            kqm_sb[0:D, bass.ts(t, 512)],
                        )
                    # V^T[n, c] = relu(xf^T @ wv^T + bv), j-block-major,
                    # two j-blocks per PSUM tile so the DVE relu pays the
                    # PSUM-read bubble once per 512 columns.  The second
                    # block's first matmul rides the first's bank clear
                    # (start=False overwrites where has_written is unset).
                    for s in (2 * t, 2 * t + 1):
                        vp = psB.tile([128, 2, C], F32, tag="v_ps")
                        for i in range(2):
                            for cb in range(2):
                                nc.tensor.matmul(
                                    vp[:, i, :],
                                    xchunk(cb, 128 * (2 * s + i), 128),
                                    wvt[:, cb, :],
                                    start=(i == 0 and cb == 0),
                                    stop=(cb == 1),
                                    skip_group_check=(i == 1),
                                )
                        nc.vector.scalar_tensor_tensor(
                            vt_sb[:, 2 * s : 2 * s + 2, :],
                            vp[:],
                            0.0,
                            nbvb[:],
                            op0=mybir.AluOpType.bypass,
                            op1=mybir.AluOpType.max,
                        )
                # centering: colsum -> -mean; m-chunk 0 is centered and
                # replicated first so the first E matmul can launch while
                # the remaining columns center lazily.  The centering copy
                # rides ACT (per-partition bias) and the replications get
                # their own DMA queue; high_priority front-loads the chain
                # past the mask/vm work in the scheduler.
                with tc.high_priority():
                    nc.vector.reduce_sum(
                        qsum[D : 2 * D],
                        qpart[D : 2 * D, :],
                        axis=mybir.AxisListType.X,
                    )
                    nc.scalar.mul(qneg[D : 2 * D], qsum[D : 2 * D], -1.0 / N)
                    for cols, ncols in ((0, 512), (512, M - 512)):
                        nc.vector.tensor_scalar_add(
                            qct[D : 2 * D, cols : cols + ncols],
                            kqm_sb[D : 2 * D, cols : cols + ncols],
                            qneg[D : 2 * D],
                        )
                        # three independent single-hop replications, one
                        # per DMA queue, so the qct fan-out is one DMA
                        # latency instead of a serial doubling chain
                        for h, eng in ((0, nc.scalar), (2, nc.gpsimd), (3, nc.sync)):
                            eng.dma_start(
                                qct[h * D : (h + 1) * D, cols : cols + ncols],
                                qct[D : 2 * D, cols : cols + ncols],
                            )
                # mask column build: 32 tiny PE transposes of the staged
                # bf16 pm row, then ONE [128, NB] exp.  These run while
                # the centering chain blocks the first E matmul, keeping
                # the PE warm through the projection->attention
                # transition.  mcp borrows a prologue-pool PSUM bank.
                # [128, NB, 2]: bf16 transposes must land 4-byte aligned
                # in PSUM, so each column is padded to an even pair
                mcp = psA.tile([128, NB, 2], BF16, tag="mcp", name="mcp")
                for jb in range(NB):
                    nc.tensor.transpose(
                        mcp[:, jb, 0:1],
                        kqm_sb[2 * D : 2 * D + 1, bass.ts(jb, 128)],
                        ones_rowb[2 * D : 2 * D + 1, 0:1],
                    )
                nc.scalar.activation(mask_col[:], mcp[:, :, 0], AF.Exp)

            # ---------------- main loop: attention ----------------
            with (
                tc.tile_pool(name="psE", bufs=2, space="PSUM") as psE,
                tc.tile_pool(name="psO", bufs=1, space="PSUM") as psO,
                tc.tile_pool(name="psZ", bufs=1, space="PSUM") as psZ,
                tc.tile_pool(name="psT", bufs=1, space="PSUM") as psT,
            ):

                def emit_e_half(mc, g, half):
                    # two K=32 matmuls packed into row groups; the two
                    # halves of a group use disjoint psE buffers so the E
                    # matmuls for one half overlap ACT's exp of the other
                    e_ps = psE.tile([128, 2, 512], F32, tag="e_ps", name="e_ps")
                    for i in range(2):
                        h = 2 * half + i
                        jb = 4 * g + h
                        nc.tensor.matmul(
                            e_ps[:, i, :],
                            k_rep[h * D : (h + 1) * D, bass.ts(jb, 128)],
                            qct[h * D : (h + 1) * D, bass.ts(mc, 512)],
                            start=True,
                            stop=True,
                            tile_position=(h * D, 0),
                        )
                    return e_ps

                def emit_vm():
                    # rank-1 mask correction, accumulated directly as
                    # per-partition columns (N=1 matmuls, no transposes).
                    # Cols 0:2 = vm per c-half; cols 2:34 = ones^T @
                    # mask_col, whose free-axis sum is the mask softmax
                    # denominator (all in one PSUM tile so every PE write
                    # and DVE read is ordered by tile deps).
                    vm_ps = psT.tile([128, NB + 2], F32, tag="mc_ps", name="vm_ps")
                    for jb in range(NB):
                        for h in range(2):
                            # one accumulation group for the whole
                            # bank: a second start=True on the same bank
                            # clears the other column's first contribution
                            nc.tensor.matmul(
                                vm_ps[:, h : h + 1],
                                vt_sb[:, jb, h * 128 : (h + 1) * 128],
                                mask_col[:, jb : jb + 1],
                                start=(jb == 0 and h == 0),
                                stop=False,
                            )
                    nc.tensor.matmul(
                        vm_ps[0:1, 2 : 2 + NB],
                        ones_pair_t[:, 0, 0:1],
                        mask_col[:],
                        start=False,
                        stop=True,
                        skip_group_check=True,
                    )
                    return vm_ps

                def emit_tail(mc, orw0, orw1, z_sb):
                    # lazy epilogue: zero PE instructions, sits in the DVE/
                    # ACT FIFOs one chunk later.  For the final chunk the
                    # arguments are the raw PSUM tiles (no one reuses those
                    # banks, so the staging copies are skipped).
                    rs = npool.tile([1, 512], F32, tag="rs", name="rs")
                    nc.vector.reciprocal_approx_fast(rs[:], z_sb[:])
                    rb = npool.tile([128, 512], F32, tag="rb", name="rb")
                    nc.gpsimd.partition_broadcast(rb[:], rs[:])
                    for ci, orw in ((0, orw0), (1, orw1)):
                        t_sb = opool.tile([128, 512], F32, tag="t_sb", name="t_sb")
                        nc.vector.tensor_mul(t_sb[:], orw[:], rb[:])
                        o_sb = opool.tile([128, 512], F32, tag="o_sb", name="o_sb")
                        nc.vector.tensor_scalar_add(
                            o_sb[:], t_sb[:], vm_col[:, ci : ci + 1]
                        )
                        nc.sync.dma_start(
                            out_d[ci * 128 : (ci + 1) * 128, bass.ts(mc, 512)],
                            o_sb[:],
                        )

                vm_ps = emit_vm()
                e_pending = [emit_e_half(0, 0, 0), emit_e_half(0, 0, 1)]

                def emit_maskz():
                    # mask normalization; deferred into the loop so the
                    # scheduler cannot hoist it ahead of the centering
                    # chain (vm_col is first needed at the mc0 tail)
                    nc.vector.reduce_sum(
                        ztt[:],
                        vm_ps[0:1, 2 : 2 + NB],
                        axis=mybir.AxisListType.X,
                    )
                    nc.vector.reciprocal_approx_fast(rz[:], ztt[:])
                    nc.gpsimd.partition_broadcast(rzc[:], rz[:])
                    for ci in range(2):
                        nc.vector.tensor_scalar(
                            vm_col[:, ci : ci + 1],
                            vm_ps[:, ci : ci + 1],
                            rzc[:, 0:1],
                            bv2c[:, ci : ci + 1],
                            op0=mybir.AluOpType.mult,
                            op1=mybir.AluOpType.add,
                        )

                tail_args = None
                for mc in range(NMC):
                    o_ps = [
                        psO.tile([128, 512], F32, tag=f"o_ps{ci}", name=f"o_ps{ci}")
                        for ci in range(2)
                    ]
                    z_ps = psZ.tile([1, 512], F32, tag="z_ps", name="z_ps")
                    for g in range(NG):
                        for half in range(2):
                            e_sbh = epool.tile(
                                [128, 2, 512], FP8, tag=f"e_sb{half}",
                                name=f"e_sb{half}",
                            )
                            nc.scalar.activation(
                                e_sbh[:], e_pending[half][:], AF.Exp
                            )
                            # refill this half's psE buffer for the next
                            # group (it frees when the exp above drains)
                            if g + 1 < NG:
                                e_pending[half] = emit_e_half(mc, g + 1, half)
                            elif mc + 1 < NMC:
                                e_pending[half] = emit_e_half(mc + 1, 0, half)
                            pair = 2 * g + half  # jb-pair index within mc
                            first = pair == 0
                            last = pair == 2 * NG - 1
                            jb = 2 * pair
                            for ci in range(2):
                                nc.tensor.matmul(
                                    o_ps[ci][:],
                                    vt_sb[:, jb : jb + 2, ci * 128 : (ci + 1) * 128],
                                    e_sbh[:],
                                    start=first,
                                    stop=last,
                                    perf_mode=DR,
                                )
                            # softmax denominator: ones-stationary DoubleRow
                            # matmul accumulated over the whole chunk
                            nc.tensor.matmul(
                                z_ps[:],
                                ones_pair,
                                e_sbh[:],
                                start=first,
                                stop=last,
                                perf_mode=DR,
                            )
                        if g == 1 and tail_args is not None:
                            emit_tail(*tail_args)
                            tail_args = None
                        if mc == 0 and g == 2 and half == 1:
                            emit_maskz()
                    if mc + 1 < NMC:
                        # eager PSUM-freeing copies so the next chunk's
                        # first accumulations never wait on the (lazy) tail
                        orw0 = opool.tile(
                            [128, 512], F32, tag="o_raw0", name="o_raw0"
                        )
                        nc.vector.tensor_copy(orw0[:], o_ps[0][:])
                        orw1 = opool.tile(
                            [128, 512], F32, tag="o_raw1", name="o_raw1"
                        )
                        nc.vector.tensor_copy(orw1[:], o_ps[1][:])
                        z_sb = npool.tile([1, 512], F32, tag="z_sb", name="z_sb")
                        nc.vector.tensor_copy(z_sb[:], z_ps[:])
                        tail_args = (mc, orw0, orw1, z_sb)
                    else:
                        # final chunk: feed the PSUM tiles straight into the
                        # epilogue
                        tail_args = (mc, o_ps[0], o_ps[1], z_ps)
                emit_tail(*tail_args)

    nc.compile()
    return nc


_NC_CACHE = {}


def _get_nc():
    if "nc" not in _NC_CACHE:
        _NC_CACHE["nc"] = build_nc()
    return _NC_CACHE["nc"]


def build_in_maps(x, wq, bq, wk, bk, wv, bv, wm, bm):
    x = np.ascontiguousarray(np.asarray(x, dtype=np.float32))
    xf = x.reshape(B, C, N)
    wq = np.asarray(wq, np.float32)
    wk = np.asarray(wk, np.float32)
    wv = np.asarray(wv, np.float32)
    wm = np.asarray(wm, np.float32)
    # packed K|Q|pm stationary: [p, cb, 0:32]=wk.T block, [32:64]=wq.T,
    # [64]=wm.T
    wkq = np.ascontiguousarray(
        np.concatenate(
            [
                wk.T.reshape(2, 128, D),
                wq.T.reshape(2, 128, D),
                wm.T.reshape(2, 128, 1),
            ],
            axis=2,
        ).transpose(1, 0, 2)
    ).astype(BF16_NP)
    wvt = np.ascontiguousarray(
        wv.T.reshape(2, 128, C).transpose(1, 0, 2)
    ).astype(BF16_NP)
    bv2 = np.ascontiguousarray(np.asarray(bv, np.float32).reshape(1, C))
    bv2c = np.ascontiguousarray(
        2.0 * np.asarray(bv, np.float32).reshape(2, 128).T
    )

    in_maps = []
    for core in range(N_CORES):
        b, half = divmod(core, 2)
        if half == 0:
            xin = xf[b]
        else:
            # own query half first; j-sums are permutation invariant
            xin = np.concatenate([xf[b][:, M:], xf[b][:, :M]], axis=1)
        in_maps.append(
            {
                "x": np.ascontiguousarray(xin).astype(BF16_NP),
                "wkq": wkq,
                "wvt": wvt,
                "bv": bv2,
                "bv2c": bv2c,
            }
        )
    return x, in_maps


def kernel(x, wq, bq, wk, bk, wv, bv, wm, bm):
    x, in_maps = build_in_maps(x, wq, bq, wk, bk, wv, bv, wm, bm)

    res = run_bass_kernel_spmd(_get_nc(), in_maps, list(range(N_CORES)))
    _NC_CACHE["last_results"] = res

    tissue = np.empty((B, C, N), np.float32)
    for core in range(N_CORES):
        b, half = divmod(core, 2)
        tissue[b][:, half * M : (half + 1) * M] = res.results[core]["out"]
    return x, tissue.reshape(B, C, H, W)
